# revision 23
# baseline (speedup 1.0000x reference)
"""TRN2 Bass kernel for nn_Attention_30485677867708.

Computes, for input [N=8192, D=256] and weights Q,K,V_down [D,H=128], V_up [H,D]:
    q = input @ Q; k = input @ K; v = input @ V_down
    attn = softmax(q @ k.T, axis=1)
    out  = (attn @ v) @ V_up            -> [N, D] fp32

Row-sharded SPMD over 8 NeuronCores (1024 rows each); K/V path replicated.

Per-core pipeline (v3 — fully chunk-interleaved):
  prep: interleaved with row-tile-0 scores, one 1024-key chunk at a time.
  scores: S[128 rows, 1024 keys] chunks via 3-term fp16 matmuls into PSUM;
        DVE chunk-max (negated) -> ACT exp(bias=-B_c, accum_out=sums) -> P.
  epilogue-as-stream: row tile rt's rescale (by F_c = exp(B_c - M) only; the
        1/Z normalization moves to the dE stage) and per-chunk xbar transpose
        are interleaved into row tile rt+1's chunk loop so no engine sees a
        burst. AV matmuls are emitted through a ready-queue lagging each
        transpose by 2 chunk slots. Last two row tiles form single-tile AV
        groups so the final AV rides the tail transposes.
  out: dE[rows, D] = oT.T @ V_up in fp16, scaled by 1/Z at the PSUM->SBUF
        copy (rows are partitions there), DMA out.
"""

import numpy as np
from contextlib import ExitStack

import concourse.bacc as bacc
from concourse import mybir
from concourse.tile import TileContext, add_dep_helper
from concourse.bass_utils import run_bass_kernel_spmd

f32 = mybir.dt.float32
f16 = mybir.dt.float16
EXP = mybir.ActivationFunctionType.Exp
COPY = mybir.ActivationFunctionType.Copy
MAX = mybir.AluOpType.max
MIN = mybir.AluOpType.min
ADD = mybir.AluOpType.add
AXX = mybir.AxisListType.X

N_CORES = 8
DEBUG_DUMP = False

# which engine runs the rescale of chunk c (c % 8): mostly GpSimd (the
# otherwise-idle engine), a couple on DVE to cap GpSimd serial time
RESCALE_DVE = {0, 4}


def build(N=8192, D=256, H=128, RPC=1024):
    """Build the per-core SPMD program. RPC = rows per core."""
    CHUNK = 1024                  # keys per softmax chunk (2 psum banks)
    NCH = N // CHUNK
    RT = RPC // 128               # row tiles per core
    NKT = N // 128                # key tiles
    KPC = CHUNK // 128            # key tiles per chunk
    KB = 512                      # matmul moving width
    CPH = max(NCH // 2, 1)        # chunks per transpose half (paired groups)
    AV_LAG = 2                    # chunk slots between transpose and its AV

    # groups: paired row tiles for rts 0..RT-3, singles for the last two.
    n_pair = max((RT - 2) // 2, 0)
    GROUPS = [[2 * g, 2 * g + 1] for g in range(n_pair)]
    GROUPS += [[RT - 2], [RT - 1]]
    MAXRT_TO_G = {max(rts): gi for gi, rts in enumerate(GROUPS)}
    RT_TO_G = {rt: gi for gi, rts in enumerate(GROUPS) for rt in rts}

    nc = bacc.Bacc("TRN2", target_bir_lowering=False)

    # per-core input is host-rotated along keys so this core's own rows are
    # the first RPC columns (softmax over keys is permutation-invariant).
    inh = nc.dram_tensor("inh", [D, N], f16, kind="ExternalInput")
    inl = nc.dram_tensor("inl", [D, N], f16, kind="ExternalInput")
    # [Qh | Ql | Kh | Kl | Vdh] each [D, H]
    wpk = nc.dram_tensor("wpk", [D, 5 * H], f16, kind="ExternalInput")
    vup = nc.dram_tensor("vup", [H, D], f16, kind="ExternalInput")
    out = nc.dram_tensor("out", [RPC, D], f32, kind="ExternalOutput")

    # Matmuls that share a PSUM zero region (bank) across separate start/stop
    # sequences must not interleave on PE: chain those explicitly.
    chain_last = [None]

    def mm(*args, chain=False, **kw):
        inst = nc.tensor.matmul(*args, **kw)
        if chain:
            if chain_last[0] is not None:
                add_dep_helper(
                    inst.ins, chain_last[0].ins, sync=False, reason="bank-order"
                )
            chain_last[0] = inst
        return inst

    with TileContext(nc) as tc, ExitStack() as ctx:
        wp = ctx.enter_context(tc.tile_pool(name="wp", bufs=1))
        big = ctx.enter_context(tc.tile_pool(name="big", bufs=1))
        ppool = ctx.enter_context(tc.tile_pool(name="ppool", bufs=3))
        smalls = ctx.enter_context(tc.tile_pool(name="smalls", bufs=4))
        rpool = ctx.enter_context(tc.tile_pool(name="rpool", bufs=RT))
        ostr = ctx.enter_context(tc.tile_pool(name="ostr", bufs=3))
        spsum = ctx.enter_context(tc.tile_pool(name="spsum", bufs=3, space="PSUM"))

        wp0 = wp.tile([128, 5 * H], f16, tag="wp0")
        wp1 = wp.tile([128, 5 * H], f16, tag="wp1")
        vu = wp.tile([H, D], f16, tag="vu")
        nc.sync.dma_start(wp0[:], wpk[0:128, :])
        nc.sync.dma_start(wp1[:], wpk[128:256, :])
        nc.sync.dma_start(vu[:], vup[:])

        kh = big.tile([128, N], f16, tag="kh")
        kl = big.tile([128, N], f16, tag="kl")
        vsb = big.tile([128, N], f16, tag="vsb")
        qh = big.tile([128, RPC], f16, tag="qh")
        ql = big.tile([128, RPC], f16, tag="ql")

        wslice = lambda c, i: (wp0 if c == 0 else wp1)[:, i * H : (i + 1) * H]

        # ---------------- per-row-tile state ----------------
        dbg_stash = {}
        P_tiles = {}
        negB_tiles = {}
        sums_tiles = {}
        F_tiles = {}
        R_tiles = {}
        pts_tiles = {}
        oab_tiles = {}

        def softmax_chunk(rt, c):
            """Scores + chunk max + exp for (row tile rt, key chunk c)."""
            if c == 0:
                P_tiles[rt] = ppool.tile([128, N], f16, tag="P", name=f"P{rt}")
                negB_tiles[rt] = smalls.tile([128, NCH], f32, tag="negB",
                                             name=f"negB{rt}")
                sums_tiles[rt] = smalls.tile([128, NCH], f32, tag="sums",
                                             name=f"sums{rt}")
            P = P_tiles[rt]
            lh = qh[:, rt * 128 : (rt + 1) * 128]
            ll = ql[:, rt * 128 : (rt + 1) * 128]
            ps = spsum.tile([128, CHUNK], f32, tag="ps")
            for hblk in range(CHUNK // KB):
                o = ps[:, hblk * KB : (hblk + 1) * KB]
                ks = slice(c * CHUNK + hblk * KB, c * CHUNK + (hblk + 1) * KB)
                mm(o, lh, kh[:, ks], start=True, stop=False)
                mm(o, lh, kl[:, ks], start=False, stop=False)
                mm(o, ll, kh[:, ks], start=False, stop=True)
            nc.vector.tensor_reduce(
                negB_tiles[rt][:, c : c + 1], ps[:], axis=AXX, op=MAX,
                negate=True,
            )
            nc.scalar.activation(
                P[:, c * CHUNK : (c + 1) * CHUNK],
                ps[:],
                EXP,
                bias=negB_tiles[rt][:, c : c + 1],
                scale=1.0,
                accum_out=sums_tiles[rt][:, c : c + 1],
            )

        def negMF(rt):
            """Global (row-tile) max and per-chunk rescale factors."""
            negB = negB_tiles[rt]
            negM = smalls.tile([128, 1], f32, tag="negM")
            nc.vector.tensor_reduce(negM[:], negB[:], axis=AXX, op=MIN)
            F = smalls.tile([128, NCH], f32, tag="F", name=f"F{rt}")
            nc.scalar.activation(F[:], negB[:], EXP, bias=negM[:], scale=-1.0)
            F_tiles[rt] = F

        def transpose_chunk(rt, c, queue):
            """Xbar-transpose P chunk c of row tile rt into its pts tile."""
            g = RT_TO_G[rt]
            rts = GROUPS[g]
            j = rts.index(rt)
            if len(rts) == 2:
                h = c // CPH
                if (g, h) not in pts_tiles:
                    pts_tiles[(g, h)] = ptsb.tile(
                        [128, 2, CPH * KPC, 128], f16,
                        tag=f"ptsP{h}", bufs=1, name=f"pts_{g}_{h}",
                    )
                pts = pts_tiles[(g, h)]
                lc = c % CPH
                dst = pts[:][:, j, lc * KPC : (lc + 1) * KPC, :]
            else:
                if (g, 0) not in pts_tiles:
                    pts_tiles[(g, 0)] = ptsb.tile(
                        [128, NKT, 128], f16, tag="ptsS", bufs=2,
                        name=f"pts_{g}_s",
                    )
                pts = pts_tiles[(g, 0)]
                dst = pts[:][:, c * KPC : (c + 1) * KPC, :]
            queue.dma_start(
                dst,
                P_tiles[rt][:, c * CHUNK : (c + 1) * CHUNK],
                transpose=True,
            )

        def epi_chunk(rt, c, queue):
            """Rescale chunk c of row tile rt by F_c and transpose it."""
            P = P_tiles[rt]
            sl = slice(c * CHUNK, (c + 1) * CHUNK)
            F = F_tiles[rt]
            if c % 8 in RESCALE_DVE:
                nc.vector.tensor_scalar_mul(P[:, sl], P[:, sl],
                                            F[:, c : c + 1])
            else:
                nc.gpsimd.tensor_scalar_mul(P[:, sl], P[:, sl],
                                            F[:, c : c + 1])
            transpose_chunk(rt, c, queue)

        def zr(rt):
            """Row sum Z = sum_c F_c * sums_c and its reciprocal."""
            F = F_tiles.pop(rt)
            sums = sums_tiles.pop(rt)
            if DEBUG_DUMP:
                dbg_stash[f"F{rt}"] = F
                dbg_stash[f"sums{rt}"] = sums
            del negB_tiles[rt]
            T = smalls.tile([128, NCH], f32, tag="T")
            nc.vector.tensor_mul(T[:], F[:], sums[:])
            Z = smalls.tile([128, 1], f32, tag="Z")
            nc.vector.tensor_reduce(Z[:], T[:], axis=AXX, op=ADD)
            R = rpool.tile([128, 1], f32, tag="R", name=f"R{rt}")
            nc.vector.reciprocal(R[:], Z[:])
            if DEBUG_DUMP:
                dbg_stash[f"R{rt}"] = R
            R_tiles[rt] = R

        def av_slice(g, cc):
            """Emit AV matmuls for key tiles [cc*KPC, (cc+1)*KPC) of group g."""
            rts = GROUPS[g]
            W = len(rts) * 128
            if cc == 0:
                oab_tiles[g] = opsum.tile([128, 256], f32, tag="oab",
                                          name=f"oab{g}")
            oacc = oab_tiles[g][:, :W]
            for i in range(cc * KPC, (cc + 1) * KPC):
                if len(rts) == 2:
                    h = i // (NKT // 2)
                    mov = pts_tiles[(g, h)][:][:, :, i % (NKT // 2), :]
                else:
                    mov = pts_tiles[(g, 0)][:][:, i, :]
                mm(
                    oacc,
                    vsb[:, i * 128 : (i + 1) * 128],
                    mov,
                    start=(cc == 0 and i == cc * KPC),
                    stop=(cc == NCH - 1 and i == (cc + 1) * KPC - 1),
                    chain=True,
                )

        def finish_group(g):
            """oT psum -> fp16 SBUF, dE matmul, 1/Z scale, DMA out."""
            rts = GROUPS[g]
            W = len(rts) * 128
            for h in (0, 1):
                pts_tiles.pop((g, h), None)
            oab = oab_tiles.pop(g)
            oTs = ostr.tile([128, 256], f16, tag="oTs")
            nc.scalar.copy(oTs[:, :W], oab[:, :W])
            for j, rt in enumerate(rts):
                pd = dpsum.tile([128, D], f32, tag="pd")
                mm(pd[:], oTs[:, j * 128 : (j + 1) * 128], vu[:],
                   start=True, stop=True)
                dEs = ostr.tile([128, D], f32, tag="dEs")
                nc.scalar.activation(
                    dEs[:], pd[:], COPY, scale=R_tiles.pop(rt)[:]
                )
                nc.scalar.dma_start(out[rt * 128 : (rt + 1) * 128, :], dEs[:])

        # ---------------- AV ready-queue ----------------
        pos_ctr = [0]
        av_queue = []  # (ready_pos, g, cc)

        def enqueue_av(g, cc):
            av_queue.append((pos_ctr[0] + AV_LAG, g, cc))

        def emit_ready_av(limit=None):
            while av_queue and av_queue[0][0] <= (
                limit if limit is not None else pos_ctr[0]
            ):
                _, g, cc = av_queue.pop(0)
                av_slice(g, cc)
                if cc == NCH - 1:
                    finish_group(g)

        # ---------------- prep helpers (row tile 0 phase) ----------------
        def hilo3(ps_ap, w_hi_i, w_lo_i, mov_h, mov_l):
            for c in range(2):
                mm(ps_ap, wslice(c, w_hi_i), mov_h[c], start=(c == 0),
                   stop=False)
                mm(ps_ap, wslice(c, w_hi_i), mov_l[c], start=False, stop=False)
                mm(ps_ap, wslice(c, w_lo_i), mov_h[c], start=False,
                   stop=(c == 1))

        # ---------------- emission ----------------
        with ExitStack() as prep:
            ipool = prep.enter_context(tc.tile_pool(name="ipool", bufs=1))
            pps = prep.enter_context(tc.tile_pool(name="pps", bufs=2,
                                                  space="PSUM"))

            ih = [ipool.tile([128, N], f16, tag=f"ih{p}", name=f"ih{p}")
                  for p in range(2)]
            il = [ipool.tile([128, N], f16, tag=f"il{p}", name=f"il{p}")
                  for p in range(2)]
            # all input DMAs up front: they clear the sync queue (and the
            # DMA semaphore rotation) before any compute-gated transpose
            spans = [(0, 512), (512, 1024)] + [
                (c * CHUNK, (c + 1) * CHUNK) for c in range(1, NCH)
            ]
            for lo, hi in spans:
                js = slice(lo, hi)
                for p in range(2):
                    psl = slice(p * 128, (p + 1) * 128)
                    nc.sync.dma_start(ih[p][:, js], inh[psl, js])
                    nc.sync.dma_start(il[p][:, js], inl[psl, js])

            def prep_v(b):
                bs = slice(b * KB, (b + 1) * KB)
                pv = pps.tile([128, KB], f32, tag="pp")
                mm(pv[:], wslice(0, 4), ih[0][:, bs], start=True, stop=False)
                mm(pv[:], wslice(1, 4), ih[1][:, bs], start=False, stop=True)
                vts = ipool.tile([128, KB], f16, tag="vts", bufs=3,
                                 name=f"vts{b}")
                nc.scalar.copy(vts[:], pv[:])
                nc.sync.dma_start(
                    vsb[:].rearrange("p (a b) -> p a b", b=128)[
                        :, 4 * b : 4 * b + 4, :],
                    vts[:],
                    transpose=True,
                )

            def prep_k(b):
                bs = slice(b * KB, (b + 1) * KB)
                pk = pps.tile([128, KB], f32, tag="pp")
                hilo3(pk[:], 2, 3,
                      [ih[0][:, bs], ih[1][:, bs]],
                      [il[0][:, bs], il[1][:, bs]])
                nc.vector.tensor_scalar_add(kh[:, bs], pk[:], 0.0)
                nc.vector.tensor_sub(kl[:, bs], pk[:], kh[:, bs])

            QB = min(KB, RPC)
            BPC = CHUNK // KB
            for c in range(NCH):
                if c == 0:
                    for b in range(RPC // QB):
                        rs = slice(b * QB, (b + 1) * QB)
                        pq = pps.tile([128, QB], f32, tag="pp")
                        hilo3(pq[:], 0, 1,
                              [ih[0][:, rs], ih[1][:, rs]],
                              [il[0][:, rs], il[1][:, rs]])
                        nc.vector.tensor_scalar_add(qh[:, rs], pq[:], 0.0)
                        nc.vector.tensor_sub(ql[:, rs], pq[:], qh[:, rs])
                for b in range(BPC):
                    prep_v(c * BPC + b)
                for b in range(BPC):
                    prep_k(c * BPC + b)
                if c > 0:
                    softmax_chunk(0, c - 1)
            softmax_chunk(0, NCH - 1)

        # prep PSUM freed; now the AV/dE psum pools fit alongside spsum
        ptsb = ctx.enter_context(tc.tile_pool(name="ptsb", bufs=2))
        opsum = ctx.enter_context(tc.tile_pool(name="opsum", bufs=1,
                                               space="PSUM"))
        dpsum = ctx.enter_context(tc.tile_pool(name="dpsum", bufs=1,
                                               space="PSUM"))

        INTERLEAVE_EPI = True
        for rt in range(1, RT):
            negMF(rt - 1)
            if not INTERLEAVE_EPI:
                for c in range(NCH):
                    epi_chunk(rt - 1, c, nc.sync)
                    if rt - 1 in MAXRT_TO_G:
                        enqueue_av(MAXRT_TO_G[rt - 1], c)
                zr(rt - 1)
            for c in range(NCH):
                pos_ctr[0] += 1
                softmax_chunk(rt, c)
                if INTERLEAVE_EPI:
                    epi_chunk(rt - 1, c, nc.sync)
                    if rt - 1 in MAXRT_TO_G:
                        enqueue_av(MAXRT_TO_G[rt - 1], c)
                emit_ready_av()
            if INTERLEAVE_EPI:
                zr(rt - 1)

        # tail: last row tile's epilogue, transposes on both HWDGE queues
        negMF(RT - 1)
        for c in range(NCH):
            pos_ctr[0] += 1
            epi_chunk(RT - 1, c, nc.sync)
            enqueue_av(MAXRT_TO_G[RT - 1], c)
            emit_ready_av()
        zr(RT - 1)
        emit_ready_av(limit=1 << 30)

        if DEBUG_DUMP:
            dbg = {
                "d_qh": (qh, f16), "d_ql": (ql, f16), "d_kh": (kh, f16),
                "d_kl": (kl, f16),
                "d_vsb": (vsb, f16), "d_P0": (P_tiles[0], f16),
                "d_P1": (P_tiles[1], f16),
            }
            for nm, t in dbg_stash.items():
                dbg["d_" + nm] = (t, f32)
            for nm, (tile, dt) in dbg.items():
                shp = [128, tile[:].free_size()]
                dt_ = nc.dram_tensor(nm, shp, dt, kind="ExternalOutput")
                nc.sync.dma_start(dt_[:, :], tile[:])

    return nc


def _split16(x):
    hi = x.astype(np.float16)
    lo = (x - hi.astype(np.float32)).astype(np.float16)
    return hi, lo


def kernel(input, Q, K, V_down, V_up):
    input = np.asarray(input, np.float32)
    Q = np.asarray(Q, np.float32)
    K = np.asarray(K, np.float32)
    V_down = np.asarray(V_down, np.float32)
    V_up = np.asarray(V_up, np.float32)

    N, D = input.shape
    H = Q.shape[1]
    RPC = N // N_CORES

    inT = np.ascontiguousarray(input.T)  # [D, N]
    inh, inl = _split16(inT)
    Qh, Ql = _split16(Q)
    Kh, Kl = _split16(K)
    Vdh = V_down.astype(np.float16)
    wpk = np.ascontiguousarray(np.concatenate([Qh, Ql, Kh, Kl, Vdh], axis=1))
    vuh = V_up.astype(np.float16)

    nc = build(N=N, D=D, H=H, RPC=RPC)
    nc.finalize()

    in_maps = []
    for c in range(N_CORES):
        r = c * RPC
        in_maps.append(
            {
                "inh": np.ascontiguousarray(np.roll(inh, -r, axis=1)),
                "inl": np.ascontiguousarray(np.roll(inl, -r, axis=1)),
                "wpk": wpk,
                "vup": vuh,
            }
        )

    res = run_bass_kernel_spmd(nc, in_maps, core_ids=list(range(N_CORES)))
    return np.concatenate([res.results[c]["out"] for c in range(N_CORES)], axis=0)


# revision 25
# speedup vs baseline: 3.1373x; 3.1373x over previous
"""TRN2 Bass kernel for nn_Attention_30485677867708.

Computes, for input [N=8192, D=256] and weights Q,K,V_down [D,H=128], V_up [H,D]:
    q = input @ Q; k = input @ K; v = input @ V_down
    attn = softmax(q @ k.T, axis=1)
    out  = (attn @ v) @ V_up            -> [N, D] fp32

Row-sharded SPMD over 8 NeuronCores (1024 rows each); K/V path replicated.

Per-core pipeline (v3 — fully chunk-interleaved):
  prep: interleaved with row-tile-0 scores, one 1024-key chunk at a time.
  scores: S[128 rows, 1024 keys] chunks via 3-term fp16 matmuls into PSUM;
        DVE chunk-max (negated) -> ACT exp(bias=-B_c, accum_out=sums) -> P.
  epilogue-as-stream: row tile rt's rescale (by F_c = exp(B_c - M) only; the
        1/Z normalization moves to the dE stage) and per-chunk xbar transpose
        are interleaved into row tile rt+1's chunk loop so no engine sees a
        burst. AV matmuls are emitted through a ready-queue lagging each
        transpose by 2 chunk slots. Last two row tiles form single-tile AV
        groups so the final AV rides the tail transposes.
  out: dE[rows, D] = oT.T @ V_up in fp16, scaled by 1/Z at the PSUM->SBUF
        copy (rows are partitions there), DMA out.
"""

import numpy as np
from contextlib import ExitStack

import concourse.bacc as bacc
from concourse import mybir
from concourse.tile import TileContext, add_dep_helper
from concourse.bass_utils import run_bass_kernel_spmd

f32 = mybir.dt.float32
f16 = mybir.dt.float16
EXP = mybir.ActivationFunctionType.Exp
COPY = mybir.ActivationFunctionType.Copy
MAX = mybir.AluOpType.max
MIN = mybir.AluOpType.min
ADD = mybir.AluOpType.add
AXX = mybir.AxisListType.X

N_CORES = 8
DEBUG_DUMP = False

# which engine runs the rescale of chunk c (c % 8): mostly DVE (cheap fp16
# tensor_scalar), one on ACT to balance the two engines' per-row-tile load
RESCALE_DVE = {0, 1, 2, 4, 5, 6, 7}


def build(N=8192, D=256, H=128, RPC=1024):
    """Build the per-core SPMD program. RPC = rows per core."""
    CHUNK = 1024                  # keys per softmax chunk (2 psum banks)
    NCH = N // CHUNK
    RT = RPC // 128               # row tiles per core
    NKT = N // 128                # key tiles
    KPC = CHUNK // 128            # key tiles per chunk
    KB = 512                      # matmul moving width
    CPH = max(NCH // 2, 1)        # chunks per transpose half (paired groups)
    AV_LAG = 2                    # chunk slots between transpose and its AV

    # groups: paired row tiles for rts 0..RT-3, singles for the last two.
    n_pair = max((RT - 2) // 2, 0)
    GROUPS = [[2 * g, 2 * g + 1] for g in range(n_pair)]
    GROUPS += [[RT - 2], [RT - 1]]
    MAXRT_TO_G = {max(rts): gi for gi, rts in enumerate(GROUPS)}
    RT_TO_G = {rt: gi for gi, rts in enumerate(GROUPS) for rt in rts}

    nc = bacc.Bacc("TRN2", target_bir_lowering=False)

    # per-core input is host-rotated along keys so this core's own rows are
    # the first RPC columns (softmax over keys is permutation-invariant).
    inh = nc.dram_tensor("inh", [D, N], f16, kind="ExternalInput")
    inl = nc.dram_tensor("inl", [D, N], f16, kind="ExternalInput")
    # [Qh | Ql | Kh | Kl | Vdh] each [D, H]
    wpk = nc.dram_tensor("wpk", [D, 5 * H], f16, kind="ExternalInput")
    vup = nc.dram_tensor("vup", [H, D], f16, kind="ExternalInput")
    out = nc.dram_tensor("out", [RPC, D], f32, kind="ExternalOutput")

    # Matmuls that share a PSUM zero region (bank) across separate start/stop
    # sequences must not interleave on PE: chain those explicitly.
    chain_last = [None]

    def mm(*args, chain=False, **kw):
        inst = nc.tensor.matmul(*args, **kw)
        if chain:
            if chain_last[0] is not None:
                add_dep_helper(
                    inst.ins, chain_last[0].ins, sync=False, reason="bank-order"
                )
            chain_last[0] = inst
        return inst

    with TileContext(nc) as tc, ExitStack() as ctx:
        wp = ctx.enter_context(tc.tile_pool(name="wp", bufs=1))
        big = ctx.enter_context(tc.tile_pool(name="big", bufs=1))
        ppool = ctx.enter_context(tc.tile_pool(name="ppool", bufs=3))
        smalls = ctx.enter_context(tc.tile_pool(name="smalls", bufs=4))
        rpool = ctx.enter_context(tc.tile_pool(name="rpool", bufs=RT))
        ostr = ctx.enter_context(tc.tile_pool(name="ostr", bufs=3))
        spsum = ctx.enter_context(tc.tile_pool(name="spsum", bufs=3, space="PSUM"))

        wp0 = wp.tile([128, 5 * H], f16, tag="wp0")
        wp1 = wp.tile([128, 5 * H], f16, tag="wp1")
        vu = wp.tile([H, D], f16, tag="vu")
        nc.sync.dma_start(wp0[:], wpk[0:128, :])
        nc.sync.dma_start(wp1[:], wpk[128:256, :])
        nc.sync.dma_start(vu[:], vup[:])

        kh = big.tile([128, N], f16, tag="kh")
        kl = big.tile([128, N], f16, tag="kl")
        vsb = big.tile([128, N], f16, tag="vsb")
        qh = big.tile([128, RPC], f16, tag="qh")
        ql = big.tile([128, RPC], f16, tag="ql")

        wslice = lambda c, i: (wp0 if c == 0 else wp1)[:, i * H : (i + 1) * H]

        # ---------------- per-row-tile state ----------------
        dbg_stash = {}
        P_tiles = {}
        negB_tiles = {}
        sums_tiles = {}
        F_tiles = {}
        R_tiles = {}
        pts_tiles = {}
        oab_tiles = {}

        def softmax_chunk(rt, c):
            """Scores + chunk max + exp for (row tile rt, key chunk c)."""
            if c == 0:
                P_tiles[rt] = ppool.tile([128, N], f16, tag="P", name=f"P{rt}")
                negB_tiles[rt] = smalls.tile([128, NCH], f32, tag="negB",
                                             name=f"negB{rt}")
                sums_tiles[rt] = smalls.tile([128, NCH], f32, tag="sums",
                                             name=f"sums{rt}")
            P = P_tiles[rt]
            lh = qh[:, rt * 128 : (rt + 1) * 128]
            ll = ql[:, rt * 128 : (rt + 1) * 128]
            ps = spsum.tile([128, CHUNK], f32, tag="ps")
            for hblk in range(CHUNK // KB):
                o = ps[:, hblk * KB : (hblk + 1) * KB]
                ks = slice(c * CHUNK + hblk * KB, c * CHUNK + (hblk + 1) * KB)
                mm(o, lh, kh[:, ks], start=True, stop=False)
                mm(o, lh, kl[:, ks], start=False, stop=False)
                mm(o, ll, kh[:, ks], start=False, stop=True)
            nc.vector.tensor_reduce(
                negB_tiles[rt][:, c : c + 1], ps[:], axis=AXX, op=MAX,
                negate=True,
            )
            nc.scalar.activation(
                P[:, c * CHUNK : (c + 1) * CHUNK],
                ps[:],
                EXP,
                bias=negB_tiles[rt][:, c : c + 1],
                scale=1.0,
                accum_out=sums_tiles[rt][:, c : c + 1],
            )

        def negMF(rt):
            """Global (row-tile) max and per-chunk rescale factors."""
            negB = negB_tiles[rt]
            negM = smalls.tile([128, 1], f32, tag="negM")
            nc.vector.tensor_reduce(negM[:], negB[:], axis=AXX, op=MIN)
            F = smalls.tile([128, NCH], f32, tag="F", name=f"F{rt}")
            nc.scalar.activation(F[:], negB[:], EXP, bias=negM[:], scale=-1.0)
            F_tiles[rt] = F

        def transpose_chunk(rt, c, queue):
            """Xbar-transpose P chunk c of row tile rt into its pts tile."""
            g = RT_TO_G[rt]
            rts = GROUPS[g]
            j = rts.index(rt)
            if len(rts) == 2:
                h = c // CPH
                if (g, h) not in pts_tiles:
                    pts_tiles[(g, h)] = ptsb.tile(
                        [128, 2, CPH * KPC, 128], f16,
                        tag=f"ptsP{h}", bufs=1, name=f"pts_{g}_{h}",
                    )
                pts = pts_tiles[(g, h)]
                lc = c % CPH
                dst = pts[:][:, j, lc * KPC : (lc + 1) * KPC, :]
            else:
                if (g, 0) not in pts_tiles:
                    pts_tiles[(g, 0)] = ptsb.tile(
                        [128, NKT, 128], f16, tag="ptsS", bufs=2,
                        name=f"pts_{g}_s",
                    )
                pts = pts_tiles[(g, 0)]
                dst = pts[:][:, c * KPC : (c + 1) * KPC, :]
            queue.dma_start(
                dst,
                P_tiles[rt][:, c * CHUNK : (c + 1) * CHUNK],
                transpose=True,
            )

        def epi_chunk(rt, c, queue):
            """Rescale chunk c of row tile rt by F_c and transpose it."""
            P = P_tiles[rt]
            sl = slice(c * CHUNK, (c + 1) * CHUNK)
            F = F_tiles[rt]
            if c % 8 in RESCALE_DVE:
                nc.vector.tensor_scalar_mul(P[:, sl], P[:, sl],
                                            F[:, c : c + 1])
            else:
                nc.scalar.activation(P[:, sl], P[:, sl], COPY,
                                     scale=F[:, c : c + 1])
            transpose_chunk(rt, c, queue)

        def zr(rt):
            """Row sum Z = sum_c F_c * sums_c and its reciprocal."""
            F = F_tiles.pop(rt)
            sums = sums_tiles.pop(rt)
            if DEBUG_DUMP:
                dbg_stash[f"F{rt}"] = F
                dbg_stash[f"sums{rt}"] = sums
            del negB_tiles[rt]
            T = smalls.tile([128, NCH], f32, tag="T")
            nc.vector.tensor_mul(T[:], F[:], sums[:])
            Z = smalls.tile([128, 1], f32, tag="Z")
            nc.vector.tensor_reduce(Z[:], T[:], axis=AXX, op=ADD)
            R = rpool.tile([128, 1], f32, tag="R", name=f"R{rt}")
            nc.vector.reciprocal(R[:], Z[:])
            if DEBUG_DUMP:
                dbg_stash[f"R{rt}"] = R
            R_tiles[rt] = R

        def av_slice(g, cc):
            """Emit AV matmuls for key tiles [cc*KPC, (cc+1)*KPC) of group g."""
            rts = GROUPS[g]
            W = len(rts) * 128
            if cc == 0:
                oab_tiles[g] = opsum.tile([128, 256], f32, tag="oab",
                                          name=f"oab{g}")
            oacc = oab_tiles[g][:, :W]
            for i in range(cc * KPC, (cc + 1) * KPC):
                if len(rts) == 2:
                    h = i // (NKT // 2)
                    mov = pts_tiles[(g, h)][:][:, :, i % (NKT // 2), :]
                else:
                    mov = pts_tiles[(g, 0)][:][:, i, :]
                mm(
                    oacc,
                    vsb[:, i * 128 : (i + 1) * 128],
                    mov,
                    start=(cc == 0 and i == cc * KPC),
                    stop=(cc == NCH - 1 and i == (cc + 1) * KPC - 1),
                    chain=True,
                )

        def finish_group(g):
            """oT psum -> fp16 SBUF, dE matmul, 1/Z scale, DMA out."""
            rts = GROUPS[g]
            W = len(rts) * 128
            for h in (0, 1):
                pts_tiles.pop((g, h), None)
            oab = oab_tiles.pop(g)
            oTs = ostr.tile([128, 256], f16, tag="oTs")
            nc.scalar.copy(oTs[:, :W], oab[:, :W])
            for j, rt in enumerate(rts):
                pd = dpsum.tile([128, D], f32, tag="pd")
                mm(pd[:], oTs[:, j * 128 : (j + 1) * 128], vu[:],
                   start=True, stop=True)
                dEs = ostr.tile([128, D], f32, tag="dEs")
                nc.scalar.activation(
                    dEs[:], pd[:], COPY, scale=R_tiles.pop(rt)[:]
                )
                nc.scalar.dma_start(out[rt * 128 : (rt + 1) * 128, :], dEs[:])

        # ---------------- AV ready-queue ----------------
        pos_ctr = [0]
        av_queue = []  # (ready_pos, g, cc)

        def enqueue_av(g, cc):
            av_queue.append((pos_ctr[0] + AV_LAG, g, cc))

        def emit_ready_av(limit=None):
            while av_queue and av_queue[0][0] <= (
                limit if limit is not None else pos_ctr[0]
            ):
                _, g, cc = av_queue.pop(0)
                av_slice(g, cc)
                if cc == NCH - 1:
                    finish_group(g)

        # ---------------- prep helpers (row tile 0 phase) ----------------
        def hilo3(ps_ap, w_hi_i, w_lo_i, mov_h, mov_l):
            for c in range(2):
                mm(ps_ap, wslice(c, w_hi_i), mov_h[c], start=(c == 0),
                   stop=False)
                mm(ps_ap, wslice(c, w_hi_i), mov_l[c], start=False, stop=False)
                mm(ps_ap, wslice(c, w_lo_i), mov_h[c], start=False,
                   stop=(c == 1))

        # ---------------- emission ----------------
        with ExitStack() as prep:
            ipool = prep.enter_context(tc.tile_pool(name="ipool", bufs=1))
            pps = prep.enter_context(tc.tile_pool(name="pps", bufs=2,
                                                  space="PSUM"))

            ih = [ipool.tile([128, N], f16, tag=f"ih{p}", name=f"ih{p}")
                  for p in range(2)]
            il = [ipool.tile([128, N], f16, tag=f"il{p}", name=f"il{p}")
                  for p in range(2)]
            # all input DMAs up front: they clear the sync queue (and the
            # DMA semaphore rotation) before any compute-gated transpose
            spans = [(0, 512), (512, 1024)] + [
                (c * CHUNK, (c + 1) * CHUNK) for c in range(1, NCH)
            ]
            for lo, hi in spans:
                js = slice(lo, hi)
                for p in range(2):
                    psl = slice(p * 128, (p + 1) * 128)
                    nc.sync.dma_start(ih[p][:, js], inh[psl, js])
                    nc.sync.dma_start(il[p][:, js], inl[psl, js])

            def prep_v(b):
                bs = slice(b * KB, (b + 1) * KB)
                pv = pps.tile([128, KB], f32, tag="pp")
                mm(pv[:], wslice(0, 4), ih[0][:, bs], start=True, stop=False)
                mm(pv[:], wslice(1, 4), ih[1][:, bs], start=False, stop=True)
                vts = ipool.tile([128, KB], f16, tag="vts", bufs=3,
                                 name=f"vts{b}")
                nc.scalar.copy(vts[:], pv[:])
                nc.sync.dma_start(
                    vsb[:].rearrange("p (a b) -> p a b", b=128)[
                        :, 4 * b : 4 * b + 4, :],
                    vts[:],
                    transpose=True,
                )

            def prep_k(b):
                bs = slice(b * KB, (b + 1) * KB)
                pk = pps.tile([128, KB], f32, tag="pp")
                hilo3(pk[:], 2, 3,
                      [ih[0][:, bs], ih[1][:, bs]],
                      [il[0][:, bs], il[1][:, bs]])
                nc.vector.tensor_scalar_add(kh[:, bs], pk[:], 0.0)
                nc.vector.tensor_sub(kl[:, bs], pk[:], kh[:, bs])

            QB = min(KB, RPC)
            BPC = CHUNK // KB
            for c in range(NCH):
                if c == 0:
                    for b in range(RPC // QB):
                        rs = slice(b * QB, (b + 1) * QB)
                        pq = pps.tile([128, QB], f32, tag="pp")
                        hilo3(pq[:], 0, 1,
                              [ih[0][:, rs], ih[1][:, rs]],
                              [il[0][:, rs], il[1][:, rs]])
                        nc.vector.tensor_scalar_add(qh[:, rs], pq[:], 0.0)
                        nc.vector.tensor_sub(ql[:, rs], pq[:], qh[:, rs])
                for b in range(BPC):
                    prep_v(c * BPC + b)
                for b in range(BPC):
                    prep_k(c * BPC + b)
                if c > 0:
                    softmax_chunk(0, c - 1)
            softmax_chunk(0, NCH - 1)

        # prep PSUM freed; now the AV/dE psum pools fit alongside spsum
        ptsb = ctx.enter_context(tc.tile_pool(name="ptsb", bufs=2))
        opsum = ctx.enter_context(tc.tile_pool(name="opsum", bufs=1,
                                               space="PSUM"))
        dpsum = ctx.enter_context(tc.tile_pool(name="dpsum", bufs=1,
                                               space="PSUM"))

        INTERLEAVE_EPI = True
        for rt in range(1, RT):
            negMF(rt - 1)
            if not INTERLEAVE_EPI:
                for c in range(NCH):
                    epi_chunk(rt - 1, c, nc.sync)
                    if rt - 1 in MAXRT_TO_G:
                        enqueue_av(MAXRT_TO_G[rt - 1], c)
                zr(rt - 1)
            for c in range(NCH):
                pos_ctr[0] += 1
                softmax_chunk(rt, c)
                if INTERLEAVE_EPI:
                    epi_chunk(rt - 1, c, nc.sync)
                    if rt - 1 in MAXRT_TO_G:
                        enqueue_av(MAXRT_TO_G[rt - 1], c)
                emit_ready_av()
            if INTERLEAVE_EPI:
                zr(rt - 1)

        # tail: last row tile's epilogue, transposes on both HWDGE queues
        negMF(RT - 1)
        for c in range(NCH):
            pos_ctr[0] += 1
            epi_chunk(RT - 1, c, nc.sync)
            enqueue_av(MAXRT_TO_G[RT - 1], c)
            emit_ready_av()
        zr(RT - 1)
        emit_ready_av(limit=1 << 30)

        if DEBUG_DUMP:
            dbg = {
                "d_qh": (qh, f16), "d_ql": (ql, f16), "d_kh": (kh, f16),
                "d_kl": (kl, f16),
                "d_vsb": (vsb, f16), "d_P0": (P_tiles[0], f16),
                "d_P1": (P_tiles[1], f16),
            }
            for nm, t in dbg_stash.items():
                dbg["d_" + nm] = (t, f32)
            for nm, (tile, dt) in dbg.items():
                shp = [128, tile[:].free_size()]
                dt_ = nc.dram_tensor(nm, shp, dt, kind="ExternalOutput")
                nc.sync.dma_start(dt_[:, :], tile[:])

    return nc


def _split16(x):
    hi = x.astype(np.float16)
    lo = (x - hi.astype(np.float32)).astype(np.float16)
    return hi, lo


def kernel(input, Q, K, V_down, V_up):
    input = np.asarray(input, np.float32)
    Q = np.asarray(Q, np.float32)
    K = np.asarray(K, np.float32)
    V_down = np.asarray(V_down, np.float32)
    V_up = np.asarray(V_up, np.float32)

    N, D = input.shape
    H = Q.shape[1]
    RPC = N // N_CORES

    inT = np.ascontiguousarray(input.T)  # [D, N]
    inh, inl = _split16(inT)
    Qh, Ql = _split16(Q)
    Kh, Kl = _split16(K)
    Vdh = V_down.astype(np.float16)
    wpk = np.ascontiguousarray(np.concatenate([Qh, Ql, Kh, Kl, Vdh], axis=1))
    vuh = V_up.astype(np.float16)

    nc = build(N=N, D=D, H=H, RPC=RPC)
    nc.finalize()

    in_maps = []
    for c in range(N_CORES):
        r = c * RPC
        in_maps.append(
            {
                "inh": np.ascontiguousarray(np.roll(inh, -r, axis=1)),
                "inl": np.ascontiguousarray(np.roll(inl, -r, axis=1)),
                "wpk": wpk,
                "vup": vuh,
            }
        )

    res = run_bass_kernel_spmd(nc, in_maps, core_ids=list(range(N_CORES)))
    return np.concatenate([res.results[c]["out"] for c in range(N_CORES)], axis=0)


# revision 31
# speedup vs baseline: 3.4953x; 1.1141x over previous
"""TRN2 Bass kernel for nn_Attention_30485677867708.

Computes, for input [N=8192, D=256] and weights Q,K,V_down [D,H=128], V_up [H,D]:
    q = input @ Q; k = input @ K; v = input @ V_down
    attn = softmax(q @ k.T, axis=1)
    out  = (attn @ v) @ V_up            -> [N, D] fp32

Row-sharded SPMD over 8 NeuronCores (1024 rows each); K/V path replicated.

Per-core pipeline (v3 — fully chunk-interleaved):
  prep: interleaved with row-tile-0 scores, one 1024-key chunk at a time.
  scores: S[128 rows, 1024 keys] chunks via 3-term fp16 matmuls into PSUM;
        DVE chunk-max (negated) -> ACT exp(bias=-B_c, accum_out=sums) -> P.
  epilogue-as-stream: row tile rt's rescale (by F_c = exp(B_c - M) only; the
        1/Z normalization moves to the dE stage) and per-chunk xbar transpose
        are interleaved into row tile rt+1's chunk loop so no engine sees a
        burst. AV matmuls are emitted through a ready-queue lagging each
        transpose by 2 chunk slots. Last two row tiles form single-tile AV
        groups so the final AV rides the tail transposes.
  out: dE[rows, D] = oT.T @ V_up in fp16, scaled by 1/Z at the PSUM->SBUF
        copy (rows are partitions there), DMA out.
"""

import numpy as np
from contextlib import ExitStack

import concourse.bacc as bacc
from concourse import mybir
from concourse.tile import TileContext, add_dep_helper
from concourse.bass_utils import run_bass_kernel_spmd

f32 = mybir.dt.float32
f16 = mybir.dt.float16
EXP = mybir.ActivationFunctionType.Exp
COPY = mybir.ActivationFunctionType.Copy
MAX = mybir.AluOpType.max
MIN = mybir.AluOpType.min
ADD = mybir.AluOpType.add
AXX = mybir.AxisListType.X

N_CORES = 8
DEBUG_DUMP = False

# which engine runs the rescale of chunk c (c % 8): mostly DVE (cheap fp16
# tensor_scalar), one on ACT to balance the two engines' per-row-tile load
RESCALE_DVE = {0, 1, 2, 4, 5, 6, 7}


def build(N=8192, D=256, H=128, RPC=1024):
    """Build the per-core SPMD program. RPC = rows per core."""
    CHUNK = 1024                  # keys per softmax chunk (2 psum banks)
    NCH = N // CHUNK
    RT = RPC // 128               # row tiles per core
    NKT = N // 128                # key tiles
    KPC = CHUNK // 128            # key tiles per chunk
    KB = 512                      # matmul moving width
    CPH = max(NCH // 2, 1)        # chunks per transpose half (paired groups)
    AV_LAG = 2                    # chunk slots between transpose and its AV

    # groups: paired row tiles; each group's AV is spread over the two row
    # tiles following it (uniform PE load); the last pair drains in the tail.
    GROUPS = [[2 * g, 2 * g + 1] for g in range(RT // 2)]
    MAXRT_TO_G = {max(rts): gi for gi, rts in enumerate(GROUPS)}
    RT_TO_G = {rt: gi for gi, rts in enumerate(GROUPS) for rt in rts}
    KPS = 4                       # key tiles per AV slice
    NSL = NKT // KPS              # AV slices per group
    SPC = NSL // NCH              # slices enqueued per epi chunk (= 2)

    nc = bacc.Bacc("TRN2", target_bir_lowering=False)

    # per-core input is host-rotated along keys so this core's own rows are
    # the first RPC columns (softmax over keys is permutation-invariant).
    inh = nc.dram_tensor("inh", [D, N], f16, kind="ExternalInput")
    inl = nc.dram_tensor("inl", [D, N], f16, kind="ExternalInput")
    # [Qh | Ql | Kh | Kl | Vdh] each [D, H]
    wpk = nc.dram_tensor("wpk", [D, 5 * H], f16, kind="ExternalInput")
    vup = nc.dram_tensor("vup", [H, D], f16, kind="ExternalInput")
    out = nc.dram_tensor("out", [RPC, D], f32, kind="ExternalOutput")

    # Matmuls that share a PSUM zero region (bank) across separate start/stop
    # sequences must not interleave on PE: chain those explicitly.
    chain_last = [None]

    def mm(*args, chain=False, **kw):
        inst = nc.tensor.matmul(*args, **kw)
        if chain:
            if chain_last[0] is not None:
                add_dep_helper(
                    inst.ins, chain_last[0].ins, sync=False, reason="bank-order"
                )
            chain_last[0] = inst
        return inst

    with TileContext(nc) as tc, ExitStack() as ctx:
        wp = ctx.enter_context(tc.tile_pool(name="wp", bufs=1))
        big = ctx.enter_context(tc.tile_pool(name="big", bufs=1))
        ppool = ctx.enter_context(tc.tile_pool(name="ppool", bufs=3))
        smalls = ctx.enter_context(tc.tile_pool(name="smalls", bufs=4))
        rpool = ctx.enter_context(tc.tile_pool(name="rpool", bufs=RT))
        ostr = ctx.enter_context(tc.tile_pool(name="ostr", bufs=3))
        spsum = ctx.enter_context(tc.tile_pool(name="spsum", bufs=3, space="PSUM"))

        wp0 = wp.tile([128, 5 * H], f16, tag="wp0")
        wp1 = wp.tile([128, 5 * H], f16, tag="wp1")
        vu = wp.tile([H, D], f16, tag="vu")
        nc.sync.dma_start(wp0[:], wpk[0:128, :])
        nc.sync.dma_start(wp1[:], wpk[128:256, :])
        nc.sync.dma_start(vu[:], vup[:])

        kh = big.tile([128, N], f16, tag="kh")
        kl = big.tile([128, N], f16, tag="kl")
        vsb = big.tile([128, N], f16, tag="vsb")
        qh = big.tile([128, RPC], f16, tag="qh")
        ql = big.tile([128, RPC], f16, tag="ql")

        wslice = lambda c, i: (wp0 if c == 0 else wp1)[:, i * H : (i + 1) * H]

        # ---------------- per-row-tile state ----------------
        dbg_stash = {}
        P_tiles = {}
        negB_tiles = {}
        sums_tiles = {}
        F_tiles = {}
        R_tiles = {}
        pts_tiles = {}
        oab_tiles = {}

        def softmax_chunk(rt, c):
            """Scores + chunk max + exp for (row tile rt, key chunk c)."""
            if c == 0:
                P_tiles[rt] = ppool.tile([128, N], f16, tag="P", name=f"P{rt}")
                negB_tiles[rt] = smalls.tile([128, NCH], f32, tag="negB",
                                             name=f"negB{rt}")
                sums_tiles[rt] = smalls.tile([128, NCH], f32, tag="sums",
                                             name=f"sums{rt}")
            P = P_tiles[rt]
            lh = qh[:, rt * 128 : (rt + 1) * 128]
            ll = ql[:, rt * 128 : (rt + 1) * 128]
            ps = spsum.tile([128, CHUNK], f32, tag="ps")
            for hblk in range(CHUNK // KB):
                o = ps[:, hblk * KB : (hblk + 1) * KB]
                ks = slice(c * CHUNK + hblk * KB, c * CHUNK + (hblk + 1) * KB)
                mm(o, lh, kh[:, ks], start=True, stop=False)
                mm(o, lh, kl[:, ks], start=False, stop=False)
                mm(o, ll, kh[:, ks], start=False, stop=True)
            nc.vector.tensor_reduce(
                negB_tiles[rt][:, c : c + 1], ps[:], axis=AXX, op=MAX,
                negate=True,
            )
            nc.scalar.activation(
                P[:, c * CHUNK : (c + 1) * CHUNK],
                ps[:],
                EXP,
                bias=negB_tiles[rt][:, c : c + 1],
                scale=1.0,
                accum_out=sums_tiles[rt][:, c : c + 1],
            )

        def negMF(rt):
            """Global (row-tile) max and per-chunk rescale factors."""
            negB = negB_tiles[rt]
            negM = smalls.tile([128, 1], f32, tag="negM")
            nc.vector.tensor_reduce(negM[:], negB[:], axis=AXX, op=MIN)
            F = smalls.tile([128, NCH], f32, tag="F", name=f"F{rt}")
            nc.scalar.activation(F[:], negB[:], EXP, bias=negM[:], scale=-1.0)
            F_tiles[rt] = F

        def transpose_chunk(rt, c, queue):
            """Xbar-transpose P chunk c of row tile rt into its pts tile."""
            g = RT_TO_G[rt]
            rts = GROUPS[g]
            j = rts.index(rt)
            h = c // CPH
            if (g, h) not in pts_tiles:
                pts_tiles[(g, h)] = ptsb.tile(
                    [128, 2, CPH * KPC, 128], f16,
                    tag=f"ptsP{h}", bufs=2, name=f"pts_{g}_{h}",
                )
            pts = pts_tiles[(g, h)]
            lc = c % CPH
            dst = pts[:][:, j, lc * KPC : (lc + 1) * KPC, :]
            queue.dma_start(
                dst,
                P_tiles[rt][:, c * CHUNK : (c + 1) * CHUNK],
                transpose=True,
            )

        def epi_chunk(rt, c, queue):
            """Rescale chunk c of row tile rt by F_c and transpose it."""
            P = P_tiles[rt]
            sl = slice(c * CHUNK, (c + 1) * CHUNK)
            F = F_tiles[rt]
            if c % 8 in RESCALE_DVE:
                nc.vector.tensor_scalar_mul(P[:, sl], P[:, sl],
                                            F[:, c : c + 1])
            else:
                nc.scalar.activation(P[:, sl], P[:, sl], COPY,
                                     scale=F[:, c : c + 1])
            transpose_chunk(rt, c, queue)

        def zr(rt):
            """Row sum Z = sum_c F_c * sums_c and its reciprocal."""
            F = F_tiles.pop(rt)
            sums = sums_tiles.pop(rt)
            if DEBUG_DUMP:
                dbg_stash[f"F{rt}"] = F
                dbg_stash[f"sums{rt}"] = sums
            del negB_tiles[rt]
            T = smalls.tile([128, NCH], f32, tag="T")
            nc.vector.tensor_mul(T[:], F[:], sums[:])
            Z = smalls.tile([128, 1], f32, tag="Z")
            nc.vector.tensor_reduce(Z[:], T[:], axis=AXX, op=ADD)
            R = rpool.tile([128, 1], f32, tag="R", name=f"R{rt}")
            nc.vector.reciprocal(R[:], Z[:])
            if DEBUG_DUMP:
                dbg_stash[f"R{rt}"] = R
            R_tiles[rt] = R

        def av_slice(g, s):
            """Emit AV matmuls for key tiles [s*KPS, (s+1)*KPS) of group g."""
            if s == 0:
                oab_tiles[g] = opsum.tile([128, 256], f32, tag="oab",
                                          name=f"oab{g}")
            oacc = oab_tiles[g][:]
            for i in range(s * KPS, (s + 1) * KPS):
                h = i // (NKT // 2)
                mov = pts_tiles[(g, h)][:][:, :, i % (NKT // 2), :]
                mm(
                    oacc,
                    vsb[:, i * 128 : (i + 1) * 128],
                    mov,
                    start=(i == 0),
                    stop=(i == NKT - 1),
                    chain=True,
                )

        def finish_group(g):
            """oT psum -> fp16 SBUF, dE matmul, 1/Z scale, DMA out."""
            rts = GROUPS[g]
            for h in (0, 1):
                pts_tiles.pop((g, h), None)
            oab = oab_tiles.pop(g)
            oTs = ostr.tile([128, 256], f16, tag="oTs")
            nc.scalar.copy(oTs[:], oab[:])
            for j, rt in enumerate(rts):
                pd = dpsum.tile([128, D], f32, tag="pd")
                mm(pd[:], oTs[:, j * 128 : (j + 1) * 128], vu[:],
                   start=True, stop=True)
                dEs = ostr.tile([128, D], f32, tag="dEs")
                nc.scalar.activation(
                    dEs[:], pd[:], COPY, scale=R_tiles.pop(rt)[:]
                )
                nc.scalar.dma_start(out[rt * 128 : (rt + 1) * 128, :], dEs[:])

        # ---------------- AV ready-queue ----------------
        pos_ctr = [0]
        av_queue = []  # (ready_pos, g, s)

        def enqueue_av(g, c):
            for s in range(c * SPC, (c + 1) * SPC):
                av_queue.append((pos_ctr[0] + AV_LAG, g, s))

        def emit_ready_av(max_emit=1, limit=None):
            n = 0
            while av_queue and n < max_emit and av_queue[0][0] <= (
                limit if limit is not None else pos_ctr[0]
            ):
                _, g, s = av_queue.pop(0)
                av_slice(g, s)
                n += 1
                if s == NSL - 1:
                    finish_group(g)

        # ---------------- prep helpers (row tile 0 phase) ----------------
        def hilo3(ps_ap, w_hi_i, w_lo_i, mov_h, mov_l):
            for c in range(2):
                mm(ps_ap, wslice(c, w_hi_i), mov_h[c], start=(c == 0),
                   stop=False)
                mm(ps_ap, wslice(c, w_hi_i), mov_l[c], start=False, stop=False)
                mm(ps_ap, wslice(c, w_lo_i), mov_h[c], start=False,
                   stop=(c == 1))

        # ---------------- emission ----------------
        with ExitStack() as prep:
            ipool = prep.enter_context(tc.tile_pool(name="ipool", bufs=1))
            pps = prep.enter_context(tc.tile_pool(name="pps", bufs=2,
                                                  space="PSUM"))

            ih = [ipool.tile([128, N], f16, tag=f"ih{p}", name=f"ih{p}")
                  for p in range(2)]
            il = [ipool.tile([128, N], f16, tag=f"il{p}", name=f"il{p}")
                  for p in range(2)]
            # all input DMAs up front: they clear the sync queue (and the
            # DMA semaphore rotation) before any compute-gated transpose
            spans = [(0, 512), (512, 1024)] + [
                (c * CHUNK, (c + 1) * CHUNK) for c in range(1, NCH)
            ]
            for lo, hi in spans:
                js = slice(lo, hi)
                for p in range(2):
                    psl = slice(p * 128, (p + 1) * 128)
                    nc.sync.dma_start(ih[p][:, js], inh[psl, js])
                    nc.sync.dma_start(il[p][:, js], inl[psl, js])

            def prep_v(b):
                bs = slice(b * KB, (b + 1) * KB)
                pv = pps.tile([128, KB], f32, tag="pp")
                mm(pv[:], wslice(0, 4), ih[0][:, bs], start=True, stop=False)
                mm(pv[:], wslice(1, 4), ih[1][:, bs], start=False, stop=True)
                vts = ipool.tile([128, KB], f16, tag="vts", bufs=3,
                                 name=f"vts{b}")
                nc.scalar.copy(vts[:], pv[:])
                nc.sync.dma_start(
                    vsb[:].rearrange("p (a b) -> p a b", b=128)[
                        :, 4 * b : 4 * b + 4, :],
                    vts[:],
                    transpose=True,
                )

            def prep_k(b):
                bs = slice(b * KB, (b + 1) * KB)
                pk = pps.tile([128, KB], f32, tag="pp")
                hilo3(pk[:], 2, 3,
                      [ih[0][:, bs], ih[1][:, bs]],
                      [il[0][:, bs], il[1][:, bs]])
                nc.vector.tensor_scalar_add(kh[:, bs], pk[:], 0.0)
                nc.vector.tensor_sub(kl[:, bs], pk[:], kh[:, bs])

            QB = min(KB, RPC)
            BPC = CHUNK // KB
            for c in range(NCH):
                if c == 0:
                    for b in range(RPC // QB):
                        rs = slice(b * QB, (b + 1) * QB)
                        pq = pps.tile([128, QB], f32, tag="pp")
                        hilo3(pq[:], 0, 1,
                              [ih[0][:, rs], ih[1][:, rs]],
                              [il[0][:, rs], il[1][:, rs]])
                        nc.vector.tensor_scalar_add(qh[:, rs], pq[:], 0.0)
                        nc.vector.tensor_sub(ql[:, rs], pq[:], qh[:, rs])
                for b in range(BPC):
                    prep_v(c * BPC + b)
                for b in range(BPC):
                    prep_k(c * BPC + b)
                if c > 0:
                    softmax_chunk(0, c - 1)
            softmax_chunk(0, NCH - 1)

        # prep PSUM freed; now the AV/dE psum pools fit alongside spsum
        ptsb = ctx.enter_context(tc.tile_pool(name="ptsb", bufs=2))
        opsum = ctx.enter_context(tc.tile_pool(name="opsum", bufs=1,
                                               space="PSUM"))
        dpsum = ctx.enter_context(tc.tile_pool(name="dpsum", bufs=1,
                                               space="PSUM"))

        INTERLEAVE_EPI = True
        for rt in range(1, RT):
            negMF(rt - 1)
            if not INTERLEAVE_EPI:
                for c in range(NCH):
                    epi_chunk(rt - 1, c, nc.sync)
                    if rt - 1 in MAXRT_TO_G:
                        enqueue_av(MAXRT_TO_G[rt - 1], c)
                zr(rt - 1)
            for c in range(NCH):
                pos_ctr[0] += 1
                softmax_chunk(rt, c)
                if INTERLEAVE_EPI:
                    epi_chunk(rt - 1, c, nc.sync)
                    if rt - 1 in MAXRT_TO_G:
                        enqueue_av(MAXRT_TO_G[rt - 1], c)
                emit_ready_av()
            if INTERLEAVE_EPI:
                zr(rt - 1)

        # tail: last row tile's epilogue; AV slices ride the transposes
        negMF(RT - 1)
        for c in range(NCH):
            pos_ctr[0] += 1
            epi_chunk(RT - 1, c, nc.sync)
            enqueue_av(MAXRT_TO_G[RT - 1], c)
            emit_ready_av(max_emit=4)
        zr(RT - 1)
        emit_ready_av(max_emit=1 << 20, limit=1 << 30)

        if DEBUG_DUMP:
            dbg = {
                "d_qh": (qh, f16), "d_ql": (ql, f16), "d_kh": (kh, f16),
                "d_kl": (kl, f16),
                "d_vsb": (vsb, f16), "d_P0": (P_tiles[0], f16),
                "d_P1": (P_tiles[1], f16),
            }
            for nm, t in dbg_stash.items():
                dbg["d_" + nm] = (t, f32)
            for nm, (tile, dt) in dbg.items():
                shp = [128, tile[:].free_size()]
                dt_ = nc.dram_tensor(nm, shp, dt, kind="ExternalOutput")
                nc.sync.dma_start(dt_[:, :], tile[:])

    return nc


def _split16(x):
    hi = x.astype(np.float16)
    lo = (x - hi.astype(np.float32)).astype(np.float16)
    return hi, lo


def kernel(input, Q, K, V_down, V_up):
    input = np.asarray(input, np.float32)
    Q = np.asarray(Q, np.float32)
    K = np.asarray(K, np.float32)
    V_down = np.asarray(V_down, np.float32)
    V_up = np.asarray(V_up, np.float32)

    N, D = input.shape
    H = Q.shape[1]
    RPC = N // N_CORES

    inT = np.ascontiguousarray(input.T)  # [D, N]
    inh, inl = _split16(inT)
    Qh, Ql = _split16(Q)
    Kh, Kl = _split16(K)
    Vdh = V_down.astype(np.float16)
    wpk = np.ascontiguousarray(np.concatenate([Qh, Ql, Kh, Kl, Vdh], axis=1))
    vuh = V_up.astype(np.float16)

    nc = build(N=N, D=D, H=H, RPC=RPC)
    nc.finalize()

    in_maps = []
    for c in range(N_CORES):
        r = c * RPC
        in_maps.append(
            {
                "inh": np.ascontiguousarray(np.roll(inh, -r, axis=1)),
                "inl": np.ascontiguousarray(np.roll(inl, -r, axis=1)),
                "wpk": wpk,
                "vup": vuh,
            }
        )

    res = run_bass_kernel_spmd(nc, in_maps, core_ids=list(range(N_CORES)))
    return np.concatenate([res.results[c]["out"] for c in range(N_CORES)], axis=0)


# revision 34
# speedup vs baseline: 3.7363x; 1.0689x over previous
"""TRN2 Bass kernel for nn_Attention_30485677867708.

Computes, for input [N=8192, D=256] and weights Q,K,V_down [D,H=128], V_up [H,D]:
    q = input @ Q; k = input @ K; v = input @ V_down
    attn = softmax(q @ k.T, axis=1)
    out  = (attn @ v) @ V_up            -> [N, D] fp32

Row-sharded SPMD over 8 NeuronCores (1024 rows each); K/V path replicated.

Per-core pipeline (v3 — fully chunk-interleaved):
  prep: interleaved with row-tile-0 scores, one 1024-key chunk at a time.
  scores: S[128 rows, 1024 keys] chunks via 3-term fp16 matmuls into PSUM;
        DVE chunk-max (negated) -> ACT exp(bias=-B_c, accum_out=sums) -> P.
  epilogue-as-stream: row tile rt's rescale (by F_c = exp(B_c - M) only; the
        1/Z normalization moves to the dE stage) and per-chunk xbar transpose
        are interleaved into row tile rt+1's chunk loop so no engine sees a
        burst. AV matmuls are emitted through a ready-queue lagging each
        transpose by 2 chunk slots. Last two row tiles form single-tile AV
        groups so the final AV rides the tail transposes.
  out: dE[rows, D] = oT.T @ V_up in fp16, scaled by 1/Z at the PSUM->SBUF
        copy (rows are partitions there), DMA out.
"""

import numpy as np
from contextlib import ExitStack

import concourse.bacc as bacc
from concourse import mybir
from concourse.tile import TileContext, add_dep_helper
from concourse.bass_utils import run_bass_kernel_spmd

f32 = mybir.dt.float32
f16 = mybir.dt.float16
EXP = mybir.ActivationFunctionType.Exp
COPY = mybir.ActivationFunctionType.Copy
MAX = mybir.AluOpType.max
MIN = mybir.AluOpType.min
ADD = mybir.AluOpType.add
AXX = mybir.AxisListType.X

N_CORES = 8
DEBUG_DUMP = False

# which engine runs the rescale of chunk c (c % 8): mostly DVE (cheap fp16
# tensor_scalar), two on ACT to balance the two engines' per-row-tile load
RESCALE_DVE = {0, 1, 2, 4, 5, 6}


def build(N=8192, D=256, H=128, RPC=1024):
    """Build the per-core SPMD program. RPC = rows per core."""
    CHUNK = 1024                  # keys per softmax chunk (2 psum banks)
    NCH = N // CHUNK
    RT = RPC // 128               # row tiles per core
    NKT = N // 128                # key tiles
    KPC = CHUNK // 128            # key tiles per chunk
    KB = 512                      # matmul moving width
    CPH = max(NCH // 2, 1)        # chunks per transpose half (paired groups)
    AV_LAG = 2                    # chunk slots between transpose and its AV

    # groups: paired row tiles; each group's AV is spread over the two row
    # tiles following it (uniform PE load); the last pair drains in the tail.
    GROUPS = [[2 * g, 2 * g + 1] for g in range(RT // 2)]
    MAXRT_TO_G = {max(rts): gi for gi, rts in enumerate(GROUPS)}
    RT_TO_G = {rt: gi for gi, rts in enumerate(GROUPS) for rt in rts}
    KPS = 4                       # key tiles per AV slice
    NSL = NKT // KPS              # AV slices per group
    SPC = NSL // NCH              # slices enqueued per epi chunk (= 2)

    nc = bacc.Bacc("TRN2", target_bir_lowering=False)

    # per-core input is host-rotated along keys so this core's own rows are
    # the first RPC columns (softmax over keys is permutation-invariant).
    inh = nc.dram_tensor("inh", [D, N], f16, kind="ExternalInput")
    inl = nc.dram_tensor("inl", [D, N], f16, kind="ExternalInput")
    # [Qh | Ql | Kh | Kl | Vdh] each [D, H]
    wpk = nc.dram_tensor("wpk", [D, 5 * H], f16, kind="ExternalInput")
    vup = nc.dram_tensor("vup", [H, D], f16, kind="ExternalInput")
    out = nc.dram_tensor("out", [RPC, D], f32, kind="ExternalOutput")

    # Matmuls that share a PSUM zero region (bank) across separate start/stop
    # sequences must not interleave on PE: chain those explicitly.
    chain_last = [None]

    def mm(*args, chain=False, **kw):
        inst = nc.tensor.matmul(*args, **kw)
        if chain:
            if chain_last[0] is not None:
                add_dep_helper(
                    inst.ins, chain_last[0].ins, sync=False, reason="bank-order"
                )
            chain_last[0] = inst
        return inst

    with TileContext(nc) as tc, ExitStack() as ctx:
        wp = ctx.enter_context(tc.tile_pool(name="wp", bufs=1))
        big = ctx.enter_context(tc.tile_pool(name="big", bufs=1))
        ppool = ctx.enter_context(tc.tile_pool(name="ppool", bufs=3))
        smalls = ctx.enter_context(tc.tile_pool(name="smalls", bufs=4))
        rpool = ctx.enter_context(tc.tile_pool(name="rpool", bufs=RT))
        ostr = ctx.enter_context(tc.tile_pool(name="ostr", bufs=3))
        spsum = ctx.enter_context(tc.tile_pool(name="spsum", bufs=3, space="PSUM"))

        wp0 = wp.tile([128, 5 * H], f16, tag="wp0")
        wp1 = wp.tile([128, 5 * H], f16, tag="wp1")
        vu = wp.tile([H, D], f16, tag="vu")
        nc.sync.dma_start(wp0[:], wpk[0:128, :])
        nc.sync.dma_start(wp1[:], wpk[128:256, :])
        nc.sync.dma_start(vu[:], vup[:])

        kh = big.tile([128, N], f16, tag="kh")
        kl = big.tile([128, N], f16, tag="kl")
        vsb = big.tile([128, N], f16, tag="vsb")
        qh = big.tile([128, RPC], f16, tag="qh")
        ql = big.tile([128, RPC], f16, tag="ql")

        wslice = lambda c, i: (wp0 if c == 0 else wp1)[:, i * H : (i + 1) * H]

        # ---------------- per-row-tile state ----------------
        dbg_stash = {}
        P_tiles = {}
        negB_tiles = {}
        sums_tiles = {}
        F_tiles = {}
        R_tiles = {}
        pts_tiles = {}
        oab_tiles = {}

        def softmax_chunk(rt, c):
            """Scores + chunk max + exp for (row tile rt, key chunk c)."""
            if c == 0:
                P_tiles[rt] = ppool.tile([128, N], f16, tag="P", name=f"P{rt}")
                negB_tiles[rt] = smalls.tile([128, NCH], f32, tag="negB",
                                             name=f"negB{rt}")
                sums_tiles[rt] = smalls.tile([128, NCH], f32, tag="sums",
                                             name=f"sums{rt}")
            P = P_tiles[rt]
            lh = qh[:, rt * 128 : (rt + 1) * 128]
            ll = ql[:, rt * 128 : (rt + 1) * 128]
            ps = spsum.tile([128, CHUNK], f32, tag="ps")
            for hblk in range(CHUNK // KB):
                o = ps[:, hblk * KB : (hblk + 1) * KB]
                ks = slice(c * CHUNK + hblk * KB, c * CHUNK + (hblk + 1) * KB)
                mm(o, lh, kh[:, ks], start=True, stop=False)
                mm(o, lh, kl[:, ks], start=False, stop=False)
                mm(o, ll, kh[:, ks], start=False, stop=True)
            nc.vector.tensor_reduce(
                negB_tiles[rt][:, c : c + 1], ps[:], axis=AXX, op=MAX,
                negate=True,
            )
            nc.scalar.activation(
                P[:, c * CHUNK : (c + 1) * CHUNK],
                ps[:],
                EXP,
                bias=negB_tiles[rt][:, c : c + 1],
                scale=1.0,
                accum_out=sums_tiles[rt][:, c : c + 1],
            )

        def negMF(rt):
            """Global (row-tile) max and per-chunk rescale factors."""
            negB = negB_tiles[rt]
            negM = smalls.tile([128, 1], f32, tag="negM")
            nc.vector.tensor_reduce(negM[:], negB[:], axis=AXX, op=MIN)
            F = smalls.tile([128, NCH], f32, tag="F", name=f"F{rt}")
            nc.scalar.activation(F[:], negB[:], EXP, bias=negM[:], scale=-1.0)
            F_tiles[rt] = F

        def transpose_chunk(rt, c, queue):
            """Xbar-transpose P chunk c of row tile rt into its pts tile."""
            g = RT_TO_G[rt]
            rts = GROUPS[g]
            j = rts.index(rt)
            h = c // CPH
            if (g, h) not in pts_tiles:
                pts_tiles[(g, h)] = ptsb.tile(
                    [128, 2, CPH * KPC, 128], f16,
                    tag=f"ptsP{h}", bufs=2, name=f"pts_{g}_{h}",
                )
            pts = pts_tiles[(g, h)]
            lc = c % CPH
            dst = pts[:][:, j, lc * KPC : (lc + 1) * KPC, :]
            queue.dma_start(
                dst,
                P_tiles[rt][:, c * CHUNK : (c + 1) * CHUNK],
                transpose=True,
            )

        def epi_chunk(rt, c, queue):
            """Rescale chunk c of row tile rt by F_c and transpose it."""
            P = P_tiles[rt]
            sl = slice(c * CHUNK, (c + 1) * CHUNK)
            F = F_tiles[rt]
            if c % 8 in RESCALE_DVE:
                nc.vector.tensor_scalar_mul(P[:, sl], P[:, sl],
                                            F[:, c : c + 1])
            else:
                nc.scalar.activation(P[:, sl], P[:, sl], COPY,
                                     scale=F[:, c : c + 1])
            transpose_chunk(rt, c, queue)

        def zr(rt):
            """Row sum Z = sum_c F_c * sums_c and its reciprocal."""
            F = F_tiles.pop(rt)
            sums = sums_tiles.pop(rt)
            if DEBUG_DUMP:
                dbg_stash[f"F{rt}"] = F
                dbg_stash[f"sums{rt}"] = sums
            del negB_tiles[rt]
            T = smalls.tile([128, NCH], f32, tag="T")
            nc.vector.tensor_mul(T[:], F[:], sums[:])
            Z = smalls.tile([128, 1], f32, tag="Z")
            nc.vector.tensor_reduce(Z[:], T[:], axis=AXX, op=ADD)
            R = rpool.tile([128, 1], f32, tag="R", name=f"R{rt}")
            nc.vector.reciprocal(R[:], Z[:])
            if DEBUG_DUMP:
                dbg_stash[f"R{rt}"] = R
            R_tiles[rt] = R

        def av_slice(g, s):
            """Emit AV matmuls for key tiles [s*KPS, (s+1)*KPS) of group g."""
            if s == 0:
                oab_tiles[g] = opsum.tile([128, 256], f32, tag="oab",
                                          name=f"oab{g}")
            oacc = oab_tiles[g][:]
            for i in range(s * KPS, (s + 1) * KPS):
                h = i // (NKT // 2)
                mov = pts_tiles[(g, h)][:][:, :, i % (NKT // 2), :]
                mm(
                    oacc,
                    vsb[:, i * 128 : (i + 1) * 128],
                    mov,
                    start=(i == 0),
                    stop=(i == NKT - 1),
                    chain=True,
                )

        def finish_group(g):
            """oT psum -> fp16 SBUF, dE matmul, 1/Z scale, DMA out."""
            rts = GROUPS[g]
            for h in (0, 1):
                pts_tiles.pop((g, h), None)
            oab = oab_tiles.pop(g)
            oTs = ostr.tile([128, 256], f16, tag="oTs")
            nc.scalar.copy(oTs[:], oab[:])
            for j, rt in enumerate(rts):
                pd = dpsum.tile([128, D], f32, tag="pd")
                mm(pd[:], oTs[:, j * 128 : (j + 1) * 128], vu[:],
                   start=True, stop=True)
                dEs = ostr.tile([128, D], f32, tag="dEs")
                nc.scalar.activation(
                    dEs[:], pd[:], COPY, scale=R_tiles.pop(rt)[:]
                )
                nc.scalar.dma_start(out[rt * 128 : (rt + 1) * 128, :], dEs[:])

        # ---------------- AV ready-queue ----------------
        pos_ctr = [0]
        av_queue = []  # (ready_pos, g, s)

        def enqueue_av(g, c):
            for s in range(c * SPC, (c + 1) * SPC):
                av_queue.append((pos_ctr[0] + AV_LAG, g, s))

        def emit_ready_av(max_emit=1, limit=None):
            n = 0
            while av_queue and n < max_emit and av_queue[0][0] <= (
                limit if limit is not None else pos_ctr[0]
            ):
                _, g, s = av_queue.pop(0)
                av_slice(g, s)
                n += 1
                if s == NSL - 1:
                    finish_group(g)

        # ---------------- prep helpers (row tile 0 phase) ----------------
        def hilo3(ps_ap, w_hi_i, w_lo_i, mov_h, mov_l):
            for c in range(2):
                mm(ps_ap, wslice(c, w_hi_i), mov_h[c], start=(c == 0),
                   stop=False)
                mm(ps_ap, wslice(c, w_hi_i), mov_l[c], start=False, stop=False)
                mm(ps_ap, wslice(c, w_lo_i), mov_h[c], start=False,
                   stop=(c == 1))

        # ---------------- emission ----------------
        with ExitStack() as prep:
            ipool = prep.enter_context(tc.tile_pool(name="ipool", bufs=1))
            pps = prep.enter_context(tc.tile_pool(name="pps", bufs=2,
                                                  space="PSUM"))

            ih = [ipool.tile([128, N], f16, tag=f"ih{p}", name=f"ih{p}")
                  for p in range(2)]
            il = [ipool.tile([128, N], f16, tag=f"il{p}", name=f"il{p}")
                  for p in range(2)]
            # all input DMAs up front: they clear the sync queue (and the
            # DMA semaphore rotation) before any compute-gated transpose
            spans = [(0, 512), (512, 1024)] + [
                (c * CHUNK, (c + 1) * CHUNK) for c in range(1, NCH)
            ]
            for lo, hi in spans:
                js = slice(lo, hi)
                for p in range(2):
                    psl = slice(p * 128, (p + 1) * 128)
                    nc.sync.dma_start(ih[p][:, js], inh[psl, js])
                    nc.sync.dma_start(il[p][:, js], inl[psl, js])

            def prep_v(b):
                bs = slice(b * KB, (b + 1) * KB)
                pv = pps.tile([128, KB], f32, tag="pp")
                mm(pv[:], wslice(0, 4), ih[0][:, bs], start=True, stop=False)
                mm(pv[:], wslice(1, 4), ih[1][:, bs], start=False, stop=True)
                vts = ipool.tile([128, KB], f16, tag="vts", bufs=3,
                                 name=f"vts{b}")
                nc.scalar.copy(vts[:], pv[:])
                nc.sync.dma_start(
                    vsb[:].rearrange("p (a b) -> p a b", b=128)[
                        :, 4 * b : 4 * b + 4, :],
                    vts[:],
                    transpose=True,
                )

            def prep_k(b):
                bs = slice(b * KB, (b + 1) * KB)
                pk = pps.tile([128, KB], f32, tag="pp")
                hilo3(pk[:], 2, 3,
                      [ih[0][:, bs], ih[1][:, bs]],
                      [il[0][:, bs], il[1][:, bs]])
                nc.scalar.copy(kh[:, bs], pk[:])
                nc.vector.tensor_sub(kl[:, bs], pk[:], kh[:, bs])

            QB = min(KB, RPC)
            BPC = CHUNK // KB
            for c in range(NCH):
                if c == 0:
                    for b in range(RPC // QB):
                        rs = slice(b * QB, (b + 1) * QB)
                        pq = pps.tile([128, QB], f32, tag="pp")
                        hilo3(pq[:], 0, 1,
                              [ih[0][:, rs], ih[1][:, rs]],
                              [il[0][:, rs], il[1][:, rs]])
                        nc.scalar.copy(qh[:, rs], pq[:])
                        nc.vector.tensor_sub(ql[:, rs], pq[:], qh[:, rs])
                for b in range(BPC):
                    prep_k(c * BPC + b)
                for b in range(BPC):
                    prep_v(c * BPC + b)
                if c > 0:
                    softmax_chunk(0, c - 1)
            softmax_chunk(0, NCH - 1)

        # prep PSUM freed; now the AV/dE psum pools fit alongside spsum
        ptsb = ctx.enter_context(tc.tile_pool(name="ptsb", bufs=2))
        opsum = ctx.enter_context(tc.tile_pool(name="opsum", bufs=1,
                                               space="PSUM"))
        dpsum = ctx.enter_context(tc.tile_pool(name="dpsum", bufs=1,
                                               space="PSUM"))

        INTERLEAVE_EPI = True
        for rt in range(1, RT):
            negMF(rt - 1)
            if not INTERLEAVE_EPI:
                for c in range(NCH):
                    epi_chunk(rt - 1, c, nc.sync)
                    if rt - 1 in MAXRT_TO_G:
                        enqueue_av(MAXRT_TO_G[rt - 1], c)
                zr(rt - 1)
            for c in range(NCH):
                pos_ctr[0] += 1
                softmax_chunk(rt, c)
                if INTERLEAVE_EPI:
                    epi_chunk(rt - 1, c, nc.sync)
                    if rt - 1 in MAXRT_TO_G:
                        enqueue_av(MAXRT_TO_G[rt - 1], c)
                emit_ready_av()
            if INTERLEAVE_EPI:
                zr(rt - 1)

        # tail: last row tile's epilogue; AV slices ride the transposes
        negMF(RT - 1)
        for c in range(NCH):
            pos_ctr[0] += 1
            epi_chunk(RT - 1, c, nc.sync)
            enqueue_av(MAXRT_TO_G[RT - 1], c)
            emit_ready_av(max_emit=4)
        zr(RT - 1)
        emit_ready_av(max_emit=1 << 20, limit=1 << 30)

        if DEBUG_DUMP:
            dbg = {
                "d_qh": (qh, f16), "d_ql": (ql, f16), "d_kh": (kh, f16),
                "d_kl": (kl, f16),
                "d_vsb": (vsb, f16), "d_P0": (P_tiles[0], f16),
                "d_P1": (P_tiles[1], f16),
            }
            for nm, t in dbg_stash.items():
                dbg["d_" + nm] = (t, f32)
            for nm, (tile, dt) in dbg.items():
                shp = [128, tile[:].free_size()]
                dt_ = nc.dram_tensor(nm, shp, dt, kind="ExternalOutput")
                nc.sync.dma_start(dt_[:, :], tile[:])

    return nc


def _split16(x):
    hi = x.astype(np.float16)
    lo = (x - hi.astype(np.float32)).astype(np.float16)
    return hi, lo


def kernel(input, Q, K, V_down, V_up):
    input = np.asarray(input, np.float32)
    Q = np.asarray(Q, np.float32)
    K = np.asarray(K, np.float32)
    V_down = np.asarray(V_down, np.float32)
    V_up = np.asarray(V_up, np.float32)

    N, D = input.shape
    H = Q.shape[1]
    RPC = N // N_CORES

    inT = np.ascontiguousarray(input.T)  # [D, N]
    inh, inl = _split16(inT)
    Qh, Ql = _split16(Q)
    Kh, Kl = _split16(K)
    Vdh = V_down.astype(np.float16)
    wpk = np.ascontiguousarray(np.concatenate([Qh, Ql, Kh, Kl, Vdh], axis=1))
    vuh = V_up.astype(np.float16)

    nc = build(N=N, D=D, H=H, RPC=RPC)
    nc.finalize()

    in_maps = []
    for c in range(N_CORES):
        r = c * RPC
        in_maps.append(
            {
                "inh": np.ascontiguousarray(np.roll(inh, -r, axis=1)),
                "inl": np.ascontiguousarray(np.roll(inl, -r, axis=1)),
                "wpk": wpk,
                "vup": vuh,
            }
        )

    res = run_bass_kernel_spmd(nc, in_maps, core_ids=list(range(N_CORES)))
    return np.concatenate([res.results[c]["out"] for c in range(N_CORES)], axis=0)


# revision 39
# speedup vs baseline: 3.7989x; 1.0167x over previous
"""TRN2 Bass kernel for nn_Attention_30485677867708.

Computes, for input [N=8192, D=256] and weights Q,K,V_down [D,H=128], V_up [H,D]:
    q = input @ Q; k = input @ K; v = input @ V_down
    attn = softmax(q @ k.T, axis=1)
    out  = (attn @ v) @ V_up            -> [N, D] fp32

Row-sharded SPMD over 8 NeuronCores (1024 rows each); K/V path replicated.

Per-core pipeline (v3 — fully chunk-interleaved):
  prep: interleaved with row-tile-0 scores, one 1024-key chunk at a time.
  scores: S[128 rows, 1024 keys] chunks via 3-term fp16 matmuls into PSUM;
        DVE chunk-max (negated) -> ACT exp(bias=-B_c, accum_out=sums) -> P.
  epilogue-as-stream: row tile rt's rescale (by F_c = exp(B_c - M) only; the
        1/Z normalization moves to the dE stage) and per-chunk xbar transpose
        are interleaved into row tile rt+1's chunk loop so no engine sees a
        burst. AV matmuls are emitted through a ready-queue lagging each
        transpose by 2 chunk slots. Last two row tiles form single-tile AV
        groups so the final AV rides the tail transposes.
  out: dE[rows, D] = oT.T @ V_up in fp16, scaled by 1/Z at the PSUM->SBUF
        copy (rows are partitions there), DMA out.
"""

import numpy as np
from contextlib import ExitStack

import concourse.bacc as bacc
from concourse import mybir
from concourse.tile import TileContext, add_dep_helper
from concourse.bass_utils import run_bass_kernel_spmd

f32 = mybir.dt.float32
f16 = mybir.dt.float16
EXP = mybir.ActivationFunctionType.Exp
COPY = mybir.ActivationFunctionType.Copy
MAX = mybir.AluOpType.max
MIN = mybir.AluOpType.min
ADD = mybir.AluOpType.add
AXX = mybir.AxisListType.X

N_CORES = 8
DEBUG_DUMP = False

# which engine runs the rescale of chunk c (c % 8): mostly DVE (cheap fp16
# tensor_scalar), one on ACT to balance the two engines' per-row-tile load
RESCALE_DVE = {0, 1, 2, 4, 5, 6, 7}


def build(N=8192, D=256, H=128, RPC=1024):
    """Build the per-core SPMD program. RPC = rows per core."""
    CHUNK = 1024                  # keys per softmax chunk (2 psum banks)
    NCH = N // CHUNK
    RT = RPC // 128               # row tiles per core
    NKT = N // 128                # key tiles
    KPC = CHUNK // 128            # key tiles per chunk
    KB = 512                      # matmul moving width
    CPH = max(NCH // 2, 1)        # chunks per transpose half (paired groups)
    AV_LAG = 2                    # chunk slots between transpose and its AV

    # groups: paired row tiles; each group's AV is spread over the two row
    # tiles following it (uniform PE load); the last pair drains in the tail.
    GROUPS = [[2 * g, 2 * g + 1] for g in range(RT // 2)]
    MAXRT_TO_G = {max(rts): gi for gi, rts in enumerate(GROUPS)}
    RT_TO_G = {rt: gi for gi, rts in enumerate(GROUPS) for rt in rts}
    KPS = 8                       # key tiles per AV slice
    NSL = NKT // KPS              # AV slices per group
    SPC = max(NSL // NCH, 1)      # slices enqueued per epi chunk

    nc = bacc.Bacc("TRN2", target_bir_lowering=False)

    # per-core input is host-rotated along keys so this core's own rows are
    # the first RPC columns (softmax over keys is permutation-invariant).
    inh = nc.dram_tensor("inh", [D, N], f16, kind="ExternalInput")
    inl = nc.dram_tensor("inl", [D, N], f16, kind="ExternalInput")
    # [Qh | Ql | Kh | Kl | Vdh] each [D, H]
    wpk = nc.dram_tensor("wpk", [D, 5 * H], f16, kind="ExternalInput")
    vup = nc.dram_tensor("vup", [H, D], f16, kind="ExternalInput")
    out = nc.dram_tensor("out", [RPC, D], f32, kind="ExternalOutput")

    # Matmuls that share a PSUM zero region (bank) across separate start/stop
    # sequences must not interleave on PE: chain those explicitly.
    chain_last = [None]

    def mm(*args, chain=False, **kw):
        inst = nc.tensor.matmul(*args, **kw)
        if chain:
            if chain_last[0] is not None:
                add_dep_helper(
                    inst.ins, chain_last[0].ins, sync=False, reason="bank-order"
                )
            chain_last[0] = inst
        return inst

    with TileContext(nc) as tc, ExitStack() as ctx:
        wp = ctx.enter_context(tc.tile_pool(name="wp", bufs=1))
        big = ctx.enter_context(tc.tile_pool(name="big", bufs=1))
        ppool = ctx.enter_context(tc.tile_pool(name="ppool", bufs=3))
        smalls = ctx.enter_context(tc.tile_pool(name="smalls", bufs=4))
        rpool = ctx.enter_context(tc.tile_pool(name="rpool", bufs=RT))
        ostr = ctx.enter_context(tc.tile_pool(name="ostr", bufs=3))
        spsum = ctx.enter_context(tc.tile_pool(name="spsum", bufs=3, space="PSUM"))

        wp0 = wp.tile([128, 5 * H], f16, tag="wp0")
        wp1 = wp.tile([128, 5 * H], f16, tag="wp1")
        vu = wp.tile([H, D], f16, tag="vu")
        nc.sync.dma_start(wp0[:], wpk[0:128, :])
        nc.sync.dma_start(wp1[:], wpk[128:256, :])
        nc.sync.dma_start(vu[:], vup[:])

        kh = big.tile([128, N], f16, tag="kh")
        kl = big.tile([128, N], f16, tag="kl")
        vsb = big.tile([128, N], f16, tag="vsb")
        qh = big.tile([128, RPC], f16, tag="qh")
        ql = big.tile([128, RPC], f16, tag="ql")

        wslice = lambda c, i: (wp0 if c == 0 else wp1)[:, i * H : (i + 1) * H]

        # ---------------- per-row-tile state ----------------
        dbg_stash = {}
        P_tiles = {}
        negB_tiles = {}
        sums_tiles = {}
        F_tiles = {}
        R_tiles = {}
        pts_tiles = {}
        oab_tiles = {}

        def softmax_chunk(rt, c):
            """Scores + chunk max + exp for (row tile rt, key chunk c)."""
            if c == 0:
                P_tiles[rt] = ppool.tile([128, N], f16, tag="P", name=f"P{rt}")
                negB_tiles[rt] = smalls.tile([128, NCH], f32, tag="negB",
                                             name=f"negB{rt}")
                sums_tiles[rt] = smalls.tile([128, NCH], f32, tag="sums",
                                             name=f"sums{rt}")
            P = P_tiles[rt]
            lh = qh[:, rt * 128 : (rt + 1) * 128]
            ll = ql[:, rt * 128 : (rt + 1) * 128]
            ps = spsum.tile([128, CHUNK], f32, tag="ps")
            for hblk in range(CHUNK // KB):
                o = ps[:, hblk * KB : (hblk + 1) * KB]
                ks = slice(c * CHUNK + hblk * KB, c * CHUNK + (hblk + 1) * KB)
                mm(o, lh, kh[:, ks], start=True, stop=False)
                mm(o, lh, kl[:, ks], start=False, stop=False)
                mm(o, ll, kh[:, ks], start=False, stop=True)
            nc.vector.tensor_reduce(
                negB_tiles[rt][:, c : c + 1], ps[:], axis=AXX, op=MAX,
                negate=True,
            )
            nc.scalar.activation(
                P[:, c * CHUNK : (c + 1) * CHUNK],
                ps[:],
                EXP,
                bias=negB_tiles[rt][:, c : c + 1],
                scale=1.0,
                accum_out=sums_tiles[rt][:, c : c + 1],
            )

        def negMF(rt):
            """Global (row-tile) max and per-chunk rescale factors."""
            negB = negB_tiles[rt]
            negM = smalls.tile([128, 1], f32, tag="negM")
            nc.vector.tensor_reduce(negM[:], negB[:], axis=AXX, op=MIN)
            F = smalls.tile([128, NCH], f32, tag="F", name=f"F{rt}")
            nc.scalar.activation(F[:], negB[:], EXP, bias=negM[:], scale=-1.0)
            F_tiles[rt] = F

        def transpose_chunk(rt, c, queue):
            """Xbar-transpose P chunk c of row tile rt into its pts tile."""
            g = RT_TO_G[rt]
            rts = GROUPS[g]
            j = rts.index(rt)
            h = c // CPH
            if (g, h) not in pts_tiles:
                pts_tiles[(g, h)] = ptsb.tile(
                    [128, 2, CPH * KPC, 128], f16,
                    tag=f"ptsP{h}", bufs=2, name=f"pts_{g}_{h}",
                )
            pts = pts_tiles[(g, h)]
            lc = c % CPH
            dst = pts[:][:, j, lc * KPC : (lc + 1) * KPC, :]
            queue.dma_start(
                dst,
                P_tiles[rt][:, c * CHUNK : (c + 1) * CHUNK],
                transpose=True,
            )

        def epi_chunk(rt, c, queue):
            """Rescale chunk c of row tile rt by F_c and transpose it."""
            P = P_tiles[rt]
            sl = slice(c * CHUNK, (c + 1) * CHUNK)
            F = F_tiles[rt]
            if c % 8 in RESCALE_DVE:
                nc.vector.tensor_scalar_mul(P[:, sl], P[:, sl],
                                            F[:, c : c + 1])
            else:
                nc.scalar.activation(P[:, sl], P[:, sl], COPY,
                                     scale=F[:, c : c + 1])
            transpose_chunk(rt, c, queue)

        def zr(rt):
            """Row sum Z = sum_c F_c * sums_c and its reciprocal."""
            F = F_tiles.pop(rt)
            sums = sums_tiles.pop(rt)
            if DEBUG_DUMP:
                dbg_stash[f"F{rt}"] = F
                dbg_stash[f"sums{rt}"] = sums
            del negB_tiles[rt]
            T = smalls.tile([128, NCH], f32, tag="T")
            nc.vector.tensor_mul(T[:], F[:], sums[:])
            Z = smalls.tile([128, 1], f32, tag="Z")
            nc.vector.tensor_reduce(Z[:], T[:], axis=AXX, op=ADD)
            R = rpool.tile([128, 1], f32, tag="R", name=f"R{rt}")
            nc.vector.reciprocal(R[:], Z[:])
            if DEBUG_DUMP:
                dbg_stash[f"R{rt}"] = R
            R_tiles[rt] = R

        def av_slice(g, s):
            """Emit AV matmuls for key tiles [s*KPS, (s+1)*KPS) of group g."""
            if s == 0:
                oab_tiles[g] = opsum.tile([128, 256], f32, tag="oab",
                                          name=f"oab{g}")
            oacc = oab_tiles[g][:]
            for i in range(s * KPS, (s + 1) * KPS):
                h = i // (NKT // 2)
                mov = pts_tiles[(g, h)][:][:, :, i % (NKT // 2), :]
                mm(
                    oacc,
                    vsb[:, i * 128 : (i + 1) * 128],
                    mov,
                    start=(i == 0),
                    stop=(i == NKT - 1),
                    chain=True,
                )

        def finish_group(g):
            """oT psum -> fp16 SBUF, dE matmul, 1/Z scale, DMA out."""
            rts = GROUPS[g]
            for h in (0, 1):
                pts_tiles.pop((g, h), None)
            oab = oab_tiles.pop(g)
            oTs = ostr.tile([128, 256], f16, tag="oTs")
            nc.scalar.copy(oTs[:], oab[:])
            for j, rt in enumerate(rts):
                pd = dpsum.tile([128, D], f32, tag="pd")
                mm(pd[:], oTs[:, j * 128 : (j + 1) * 128], vu[:],
                   start=True, stop=True)
                dEs = ostr.tile([128, D], f32, tag="dEs")
                nc.vector.tensor_scalar_mul(dEs[:], pd[:], R_tiles.pop(rt)[:])
                nc.scalar.dma_start(out[rt * 128 : (rt + 1) * 128, :], dEs[:])

        # ---------------- AV ready-queue ----------------
        pos_ctr = [0]
        av_queue = []  # (ready_pos, g, s)

        def enqueue_av(g, c):
            # stagger readiness so a group's slices spread over the two
            # hosting row tiles (1 slice per 2 chunk slots)
            for s in range(c * SPC, (c + 1) * SPC):
                av_queue.append((pos_ctr[0] + AV_LAG + s, g, s))

        def emit_ready_av(max_emit=1, limit=None):
            n = 0
            while av_queue and n < max_emit and av_queue[0][0] <= (
                limit if limit is not None else pos_ctr[0]
            ):
                _, g, s = av_queue.pop(0)
                av_slice(g, s)
                n += 1
                if s == NSL - 1:
                    finish_group(g)

        # ---------------- prep helpers (row tile 0 phase) ----------------
        def hilo3(ps_ap, w_hi_i, w_lo_i, mov_h, mov_l):
            for c in range(2):
                mm(ps_ap, wslice(c, w_hi_i), mov_h[c], start=(c == 0),
                   stop=False)
                mm(ps_ap, wslice(c, w_hi_i), mov_l[c], start=False, stop=False)
                mm(ps_ap, wslice(c, w_lo_i), mov_h[c], start=False,
                   stop=(c == 1))

        # ---------------- emission ----------------
        with ExitStack() as prep:
            ipool = prep.enter_context(tc.tile_pool(name="ipool", bufs=1))
            pps = prep.enter_context(tc.tile_pool(name="pps", bufs=2,
                                                  space="PSUM"))

            ih = [ipool.tile([128, N], f16, tag=f"ih{p}", name=f"ih{p}")
                  for p in range(2)]
            il = [ipool.tile([128, N], f16, tag=f"il{p}", name=f"il{p}")
                  for p in range(2)]
            # all input DMAs up front: they clear the sync queue (and the
            # DMA semaphore rotation) before any compute-gated transpose
            spans = [(0, 512), (512, 1024)] + [
                (c * CHUNK, (c + 1) * CHUNK) for c in range(1, NCH)
            ]
            for lo, hi in spans:
                js = slice(lo, hi)
                for p in range(2):
                    psl = slice(p * 128, (p + 1) * 128)
                    nc.sync.dma_start(ih[p][:, js], inh[psl, js])
                    nc.sync.dma_start(il[p][:, js], inl[psl, js])

            def prep_v(b):
                bs = slice(b * KB, (b + 1) * KB)
                pv = pps.tile([128, KB], f32, tag="pp")
                mm(pv[:], wslice(0, 4), ih[0][:, bs], start=True, stop=False)
                mm(pv[:], wslice(1, 4), ih[1][:, bs], start=False, stop=True)
                vts = ipool.tile([128, KB], f16, tag="vts", bufs=3,
                                 name=f"vts{b}")
                nc.vector.tensor_scalar_add(vts[:], pv[:], 0.0)
                nc.sync.dma_start(
                    vsb[:].rearrange("p (a b) -> p a b", b=128)[
                        :, 4 * b : 4 * b + 4, :],
                    vts[:],
                    transpose=True,
                )

            def prep_k(b):
                bs = slice(b * KB, (b + 1) * KB)
                pk = pps.tile([128, KB], f32, tag="pp")
                hilo3(pk[:], 2, 3,
                      [ih[0][:, bs], ih[1][:, bs]],
                      [il[0][:, bs], il[1][:, bs]])
                nc.scalar.copy(kh[:, bs], pk[:])
                nc.vector.tensor_sub(kl[:, bs], pk[:], kh[:, bs])

            QB = min(KB, RPC)
            BPC = CHUNK // KB
            for c in range(NCH):
                if c == 0:
                    for b in range(RPC // QB):
                        rs = slice(b * QB, (b + 1) * QB)
                        pq = pps.tile([128, QB], f32, tag="pp")
                        hilo3(pq[:], 0, 1,
                              [ih[0][:, rs], ih[1][:, rs]],
                              [il[0][:, rs], il[1][:, rs]])
                        nc.scalar.copy(qh[:, rs], pq[:])
                        nc.vector.tensor_sub(ql[:, rs], pq[:], qh[:, rs])
                for b in range(BPC):
                    prep_k(c * BPC + b)
                for b in range(BPC):
                    prep_v(c * BPC + b)
                if c > 0:
                    softmax_chunk(0, c - 1)
            softmax_chunk(0, NCH - 1)

        # prep PSUM freed; now the AV/dE psum pools fit alongside spsum
        ptsb = ctx.enter_context(tc.tile_pool(name="ptsb", bufs=2))
        opsum = ctx.enter_context(tc.tile_pool(name="opsum", bufs=1,
                                               space="PSUM"))
        dpsum = ctx.enter_context(tc.tile_pool(name="dpsum", bufs=1,
                                               space="PSUM"))

        INTERLEAVE_EPI = True
        for rt in range(1, RT):
            negMF(rt - 1)
            if not INTERLEAVE_EPI:
                for c in range(NCH):
                    epi_chunk(rt - 1, c, nc.sync)
                    if rt - 1 in MAXRT_TO_G:
                        enqueue_av(MAXRT_TO_G[rt - 1], c)
                zr(rt - 1)
            for c in range(NCH):
                pos_ctr[0] += 1
                softmax_chunk(rt, c)
                if INTERLEAVE_EPI:
                    epi_chunk(rt - 1, c, nc.sync)
                    if rt - 1 in MAXRT_TO_G:
                        enqueue_av(MAXRT_TO_G[rt - 1], c)
                emit_ready_av()
            if INTERLEAVE_EPI:
                zr(rt - 1)

        # tail: last row tile's epilogue; AV slices ride the transposes
        negMF(RT - 1)
        for c in range(NCH):
            pos_ctr[0] += 1
            epi_chunk(RT - 1, c, nc.sync)
            enqueue_av(MAXRT_TO_G[RT - 1], c)
            emit_ready_av(max_emit=4)
        zr(RT - 1)
        emit_ready_av(max_emit=1 << 20, limit=1 << 30)

        if DEBUG_DUMP:
            dbg = {
                "d_qh": (qh, f16), "d_ql": (ql, f16), "d_kh": (kh, f16),
                "d_kl": (kl, f16),
                "d_vsb": (vsb, f16), "d_P0": (P_tiles[0], f16),
                "d_P1": (P_tiles[1], f16),
            }
            for nm, t in dbg_stash.items():
                dbg["d_" + nm] = (t, f32)
            for nm, (tile, dt) in dbg.items():
                shp = [128, tile[:].free_size()]
                dt_ = nc.dram_tensor(nm, shp, dt, kind="ExternalOutput")
                nc.sync.dma_start(dt_[:, :], tile[:])

    return nc


def _split16(x):
    hi = x.astype(np.float16)
    lo = (x - hi.astype(np.float32)).astype(np.float16)
    return hi, lo


def kernel(input, Q, K, V_down, V_up):
    input = np.asarray(input, np.float32)
    Q = np.asarray(Q, np.float32)
    K = np.asarray(K, np.float32)
    V_down = np.asarray(V_down, np.float32)
    V_up = np.asarray(V_up, np.float32)

    N, D = input.shape
    H = Q.shape[1]
    RPC = N // N_CORES

    inT = np.ascontiguousarray(input.T)  # [D, N]
    inh, inl = _split16(inT)
    Qh, Ql = _split16(Q)
    Kh, Kl = _split16(K)
    Vdh = V_down.astype(np.float16)
    wpk = np.ascontiguousarray(np.concatenate([Qh, Ql, Kh, Kl, Vdh], axis=1))
    vuh = V_up.astype(np.float16)

    nc = build(N=N, D=D, H=H, RPC=RPC)
    nc.finalize()

    in_maps = []
    for c in range(N_CORES):
        r = c * RPC
        in_maps.append(
            {
                "inh": np.ascontiguousarray(np.roll(inh, -r, axis=1)),
                "inl": np.ascontiguousarray(np.roll(inl, -r, axis=1)),
                "wpk": wpk,
                "vup": vuh,
            }
        )

    res = run_bass_kernel_spmd(nc, in_maps, core_ids=list(range(N_CORES)))
    return np.concatenate([res.results[c]["out"] for c in range(N_CORES)], axis=0)
